# revision 8
# baseline (speedup 1.0000x reference)
"""Trainium2 Bass kernel for nn_Attention_41729902248209.

8-head attention block: x (8, 512, 32, 32) -> QKV proj -> softmax attention
-> out proj + residual. Data-parallel over batch: one batch element per
NeuronCore (8 cores).

Per-core dataflow (n = 1024 tokens, cin = 512, H = 8 heads, D = 64):
  - everything stays "transposed" (feature dim on partitions) so no on-chip
    transposes are needed anywhere:
      qT, kT : (f' = 64h+d on partitions, n free)   [head pairs share a tile]
      v      : (n on partitions, 65h+d free, with a ones column per head)
      scoresT: (j on partitions, i free) = k @ qT
      pT     : exp(scoresT) in fp16 (no max subtraction; logits are O(7))
      outT~  : [v | 1].T @ pT -> (65, i) in PSUM: rows 0:64 = unscaled outT,
               row 64 = softmax denominator (ones column integrates exp)
      yT     : W_last.T.T @ outT_scaled + (x + b_last fp16)  (residual)
  - HEAD-PAIR CONCURRENT SCORES: heads 2p/2p+1 live at partitions 0:64 /
    64:128 of one f'-tile, so their 64-contraction score matmuls occupy
    disjoint PE row-groups (tile_position (0,0) vs (64,0)). Emitting the
    pair's matmuls back-to-back lets the PE run both streams concurrently,
    halving the scores wall-time. The exp stream (ACT) is the rate limiter
    of the scores phase; all other PE work (qk/v projections, attn@v) is
    drained into the per-round PE idle gap with a budget-based scheduler.
  - softmax scale 1/8 is folded into W_q host-side; b_last is folded into
    the fp16 residual xr16; b_q/b_k are per-partition DVE adds; b_v is a
    DVE tensor add.
  - denominators (per (head, chunk)): DVE reciprocal off the PSUM denom row
    (partition 64), a 1-partition cross-quadrant copy to partition 0,
    GPSIMD partition_broadcast to 128 partitions, then one DVE multiply
    straight off PSUM into outT_s (fp16). No DMAs on that chain.
  - head: PE warm-up matmuls on a zero tile defeat the HAM cold-clock
    (first ~3.4us run at 1.2 GHz otherwise) and a junk exp preloads the
    ACT function table before the first real exp needs it.
  - tail: y is fp16 (error budget is generous); final DMAs are spread
    across idle engine DGE queues.
"""

import numpy as np

import concourse.mybir as mybir
import concourse.tile as tile
from concourse import bacc
from concourse.bass_utils import run_bass_kernel_spmd

F16 = mybir.dt.float16
F32 = mybir.dt.float32

BS = 8
H = 8
D = 64
CIN = 512
N = 1024
NK = CIN // 128  # contraction tiles for cin
NJT = N // 128  # j tiles
NCH = N // 512  # i chunks of 512
VROW = H * (D + 1)  # 520: per j-tile v row: 8x[v_h (64) | 1]

AF = mybir.ActivationFunctionType
ALU = mybir.AluOpType

# drain-unit PE costs (ns, warm) for the budget scheduler
COST_V = 853
COST_QK = 853
COST_AV = 427
ROUND_BUDGET = 1220  # PE idle per score round (2 exps ~2076ns - 4 mms ~852ns)


def _emit(tc, d, sb, ps):
    nc = tc.nc

    x16_sb = sb.tile([128, NK * N], F16, tag="x16")
    xr_sb = sb.tile([128, NK * N], F16, tag="xr")
    wq_sb = sb.tile([128, NK * 512], F16, tag="wq")
    wk_sb = sb.tile([128, NK * 512], F16, tag="wk")
    wv_sb = sb.tile([128, NK * 512], F16, tag="wv")
    wl_sb = sb.tile([128, NK * 512], F16, tag="wl")
    bqk_sb = sb.tile([128, 8], F32, tag="bqk")
    bvb_sb = sb.tile([128, 512], F32, tag="bvb")
    qT_sb = sb.tile([128, 4 * N], F16, tag="qT")
    kT_sb = sb.tile([128, 4 * N], F16, tag="kT")
    v_sb = sb.tile([128, NJT * VROW], F16, tag="v")
    os_sb = sb.tile([128, 4 * N], F16, tag="outT_s")
    z_sb = sb.tile([128, 640], F16, tag="warmz")

    # --- warm-up: PE matmuls on zeros + ACT exp-table preload ---
    # (the exp reads/writes columns the PE never touches, so the table load
    # isn't serialized behind the warm matmuls)
    nc.vector.memset(z_sb[:], 0.0)
    for i in range(6):
        zp = ps.tile([128, 512], F32, tag="mm", bufs=2, name=f"warm{i}")
        nc.tensor.matmul(
            zp[0:64, :], z_sb[:, 0:64], z_sb[:, 64:576], start=True, stop=True
        )
    nc.scalar.activation(z_sb[0:128, 608:624], z_sb[:, 576:592], AF.Exp)

    # --- input DMAs (ktile k of a (512, W) dram tensor -> cols [W*k, W*k+W)) ---
    # Issue is the bottleneck (one sequencer = ~0.5-1.6us per DMA, serial per
    # queue), so spread the loads across idle engines' DGE queues.
    # (ACT queue is kept free: it is the exp bottleneck engine)
    for k in range(NK):
        r = slice(128 * k, 128 * k + 128)
        nc.sync.dma_start(wq_sb[:, 512 * k : 512 * k + 512], d["wq"].ap()[r, :])
        nc.gpsimd.dma_start(x16_sb[:, N * k : N * k + N], d["x16"].ap()[r, :])
        nc.vector.dma_start(wk_sb[:, 512 * k : 512 * k + 512], d["wk"].ap()[r, :])
    nc.sync.dma_start(bqk_sb[:], d["bqk"].ap())
    for k in range(NK):
        r = slice(128 * k, 128 * k + 128)
        nc.sync.dma_start(wv_sb[:, 512 * k : 512 * k + 512], d["wv"].ap()[r, :])
    nc.gpsimd.dma_start(bvb_sb[:], d["bvb"].ap())
    # ones columns for v~ (column 64 of each 65-wide head block)
    ones_cols = v_sb[:].rearrange("p (jt h e) -> p jt h e", jt=NJT, e=D + 1)[
        :, :, :, D : D + 1
    ]
    nc.vector.memset(ones_cols, 1.0)

    def late_dma(k):
        """wl / xr16 are only needed at ~70% of the kernel; issuing them up
        front would delay the first qT/kT evacs behind their issue cost."""
        r = slice(128 * k, 128 * k + 128)
        nc.vector.dma_start(wl_sb[:, 512 * k : 512 * k + 512], d["wl"].ap()[r, :])
        nc.gpsimd.dma_start(xr_sb[:, N * k : N * k + N], d["xr16"].ap()[r, :])

    # --- stage emitters ---
    def qk_quarter(t, wsb, dst, bcol, c):
        """One quarter of a q/k projection f'-tile: 4 mms + bias evac."""
        p = ps.tile([128, 512], F32, tag="mm", bufs=2, name=f"qk{t}_{bcol}_{c}")
        for k in range(NK):
            nc.tensor.matmul(
                p[:],
                wsb[:, 512 * k + 128 * t : 512 * k + 128 * t + 128],
                x16_sb[:, N * k + 512 * c : N * k + 512 * c + 512],
                start=(k == 0),
                stop=(k == NK - 1),
            )
        nc.vector.tensor_scalar_add(
            dst[:, N * t + 512 * c : N * t + 512 * c + 512],
            p[:],
            bqk_sb[:, bcol : bcol + 1],
        )

    def v_tile(jt):
        """Project v for token tile jt: (128 tokens, 512 feats) + b_v."""
        p = ps.tile([128, 512], F32, tag="mm", bufs=2, name=f"v{jt}")
        for k in range(NK):
            nc.tensor.matmul(
                p[:],
                x16_sb[:, N * k + 128 * jt : N * k + 128 * jt + 128],
                wv_sb[:, 512 * k : 512 * k + 512],
                start=(k == 0),
                stop=(k == NK - 1),
            )
        dst = (
            v_sb[:, VROW * jt : VROW * jt + VROW]
            .rearrange("p (h e) -> p h e", e=D + 1)[:, :, 0:D]
        )
        nc.vector.tensor_tensor(
            dst,
            p[:].rearrange("p (h e) -> p h e", e=D),
            bvb_sb[:].rearrange("p (h e) -> p h e", e=D),
            ALU.add,
        )

    pt_tiles = {}

    def scores_round(pr, jt):
        """Paired scores for heads (2pr, 2pr+1) at j-tile jt + exps.

        The two heads' stationaries occupy disjoint PE row-groups
        (partitions 0:64 vs 64:128), so interleaving their matmuls runs
        both streams concurrently on the PE array."""
        sps = []
        for hh in (0, 1):
            h = 2 * pr + hh
            if h not in pt_tiles:
                pt_tiles[h] = sbuf_pt_pool.tile(
                    [128, NJT * N], F16, tag="pt", name=f"pt{h}"
                )
            sp = ps.tile([128, N], F32, tag="score", bufs=2, name=f"s{h}_{jt}")
            sps.append(sp)
        for c in range(NCH):
            for hh in (0, 1):
                po = 64 * hh
                nc.tensor.matmul(
                    sps[hh][:, 512 * c : 512 * c + 512],
                    kT_sb[po : po + 64, N * pr + 128 * jt : N * pr + 128 * jt + 128],
                    qT_sb[po : po + 64, N * pr + 512 * c : N * pr + 512 * c + 512],
                    start=True,
                    stop=True,
                    tile_position=(po, 0),
                )
        for hh in (0, 1):
            h = 2 * pr + hh
            nc.scalar.activation(
                pt_tiles[h][:, N * jt : N * jt + N], sps[hh][:], AF.Exp
            )

    pv_tiles = {}

    def attnv_unit(h, c, g2):
        """2 accumulating matmuls (j-tiles 2*g2, 2*g2+1) of outT~ for (h, c);
        evac + denominator extraction after the last unit of the chunk."""
        pr, hh = divmod(h, 2)
        pT = pt_tiles[h]
        key = (h, c)
        if key not in pv_tiles:
            pv_tiles[key] = ps.tile([128, 512], F32, tag="av", bufs=2, name=f"av{h}_{c}")
        p = pv_tiles[key]
        for jt in (2 * g2, 2 * g2 + 1):
            nc.tensor.matmul(
                p[0:65, :],
                v_sb[:, VROW * jt + 65 * h : VROW * jt + 65 * h + 65],
                pT[:, N * jt + 512 * c : N * jt + 512 * c + 512],
                start=(jt == 0),
                stop=(jt == NJT - 1),
            )
        if g2 == 3:
            del pv_tiles[key]
            if c == NCH - 1:
                del pt_tiles[h]
            r = 2 * h + c
            # denominator chain, DMA-free: recip psum row (p64) -> sbuf f32
            # (p64), cross-quadrant copy p64 -> p0, gpsimd broadcast, then
            # one DVE multiply straight off PSUM into outT_s.
            dsl = dscr_pool.tile([128, 512], F32, tag="dscr", name=f"ds{r}")
            nc.vector.reciprocal(dsl[64:65, :], p[64:65, :])
            rrow = rr_pool.tile([1, 512], F32, tag="rrow", name=f"rr{r}")
            nc.vector.tensor_copy(rrow[0:1, :], dsl[64:65, :])
            rb = rb_pool.tile([128, 512], F32, tag="rb", name=f"rb{r}")
            nc.gpsimd.partition_broadcast(rb[:], rrow[0:1, :])
            sl = slice(N * pr + 512 * c, N * pr + 512 * c + 512)
            nc.vector.tensor_tensor(
                os_sb[64 * hh : 64 * hh + 64, sl],
                p[0:64, :],
                rb[0:64, :],
                ALU.mult,
            )

    def outproj(ct, dma_engine, split):
        """yT c-tile ct: W_lastT.T @ outT_s + xr16 residual, fp16 out + DMA.

        PSUM comes from the score tag's banks (idle once exps are done)."""
        p = ps.tile([128, N], F32, tag="score", bufs=2, name=f"yp{ct}")
        for c in range(NCH):
            for k in range(NK):
                nc.tensor.matmul(
                    p[:, 512 * c : 512 * c + 512],
                    wl_sb[:, 512 * k + 128 * ct : 512 * k + 128 * ct + 128],
                    os_sb[:, N * k + 512 * c : N * k + 512 * c + 512],
                    start=(k == 0),
                    stop=(k == NK - 1),
                )
        if not split:
            y = y_pool.tile([128, N], F16, tag="y")
            nc.vector.tensor_tensor(
                y[:], p[:], xr_sb[:, N * ct : N * ct + N], ALU.add
            )
            dma_engine.dma_start(d["y"].ap()[128 * ct : 128 * ct + 128, :], y[:])
        else:
            # split the last c-tile so its evac/DMA pipeline drains earlier
            for c, eng in zip(range(NCH), (nc.sync, nc.scalar)):
                sl = slice(512 * c, 512 * c + 512)
                y = y_pool.tile([128, 512], F16, tag="y2", name=f"y{ct}_{c}")
                nc.vector.tensor_tensor(
                    y[:], p[:, sl], xr_sb[:, N * ct + 512 * c : N * ct + 512 * c + 512],
                    ALU.add,
                )
                eng.dma_start(d["y"].ap()[128 * ct : 128 * ct + 128, sl], y[:])

    # --- pools that emitters close over ---
    import contextlib

    stack = contextlib.ExitStack()
    sbuf_pt_pool = stack.enter_context(tc.tile_pool(name="pt", bufs=6))
    rb_pool = stack.enter_context(tc.tile_pool(name="rb", bufs=3))
    rr_pool = stack.enter_context(tc.tile_pool(name="rr", bufs=3))
    y_pool = stack.enter_context(tc.tile_pool(name="y", bufs=3))
    dscr_pool = stack.enter_context(tc.tile_pool(name="dscr", bufs=2))

    # --- drain-work queue: all non-score PE work, dependency-ordered ---
    # (cost_ns, kind, emit_fn). QK quarters for f'-tile t must complete
    # before pair t's scores; AV units for pair p enter the queue only
    # after pair p's scores are emitted.
    drain_q = []

    def add_qk(t):
        for wsb, dst, bcol in ((wq_sb, qT_sb, t), (wk_sb, kT_sb, 4 + t)):
            for c in range(NCH):
                drain_q.append(
                    (COST_QK, ("qk", t),
                     lambda t=t, w=wsb, ds=dst, b=bcol, c=c: qk_quarter(t, w, ds, b, c))
                )

    def add_av_pair(pr, qks=()):
        """attnv units for heads (2pr, 2pr+1), with optional qk quarters of
        a later tile interleaved mid-pair."""
        units = [
            (COST_AV, ("av", h, c, g2), lambda h=h, c=c, g2=g2: attnv_unit(h, c, g2))
            for h in (2 * pr, 2 * pr + 1)
            for c in range(NCH)
            for g2 in range(4)
        ]
        drain_q.extend(units[:8])
        for q in qks:
            drain_q.append(q)
        drain_q.extend(units[8:])

    def drain(budget):
        spent = 0
        while drain_q and spent < budget:
            cost, _, fn = drain_q.pop(0)
            fn()
            spent += cost

    def drain_through_qk(t):
        """Force-drain until all qk quarters for f'-tile t are emitted."""
        while any(k[0] == "qk" and k[1] <= t for _, k, _ in drain_q):
            cost, _, fn = drain_q.pop(0)
            fn()

    # --- emission ---
    for wsb, dst, bcol in ((wq_sb, qT_sb, 0), (wk_sb, kT_sb, 4)):
        for c in range(NCH):
            qk_quarter(0, wsb, dst, bcol, c)

    # pair-0 era work: v tiles + qk tile 1 (v[0,1] first: earliest attnv needs)
    drain_q.append((COST_V, ("v", 0), lambda: v_tile(0)))
    drain_q.append((COST_V, ("v", 1), lambda: v_tile(1)))
    add_qk(1)
    for jt in range(2, NJT):
        drain_q.append((COST_V, ("v", jt), lambda jt=jt: v_tile(jt)))

    for pr in range(4):
        drain_through_qk(pr)
        for jt in range(NJT):
            scores_round(pr, jt)
            drain(ROUND_BUDGET)
        # queue next era's work now that pair pr's exps are all emitted
        if pr == 0:
            qk_units = []
            add_qk(2)
            qk_units = drain_q[-4:]
            del drain_q[-4:]
            add_av_pair(0, qks=qk_units)
        elif pr == 1:
            add_qk(3)
            qk_units = drain_q[-4:]
            del drain_q[-4:]
            add_av_pair(1, qks=qk_units)
        elif pr == 2:
            add_av_pair(2)
        else:
            add_av_pair(3)

    drain(1 << 30)  # remainder: attnv of pair 3 + last denom chains

    outproj(0, nc.sync, False)
    outproj(1, nc.scalar, False)
    outproj(2, nc.gpsimd, False)
    outproj(3, None, True)

    stack.close()


def _build(loop=1):
    nc = bacc.Bacc("TRN2", target_bir_lowering=False, debug=False, num_devices=BS)
    d = {}
    d["x16"] = nc.dram_tensor("x16", [CIN, N], F16, kind="ExternalInput")
    d["xr16"] = nc.dram_tensor("xr16", [CIN, N], F16, kind="ExternalInput")
    d["wq"] = nc.dram_tensor("wq", [CIN, 512], F16, kind="ExternalInput")
    d["wk"] = nc.dram_tensor("wk", [CIN, 512], F16, kind="ExternalInput")
    d["wv"] = nc.dram_tensor("wv", [CIN, 512], F16, kind="ExternalInput")
    d["wl"] = nc.dram_tensor("wl", [CIN, 512], F16, kind="ExternalInput")
    d["bqk"] = nc.dram_tensor("bqk", [128, 8], F32, kind="ExternalInput")
    d["bvb"] = nc.dram_tensor("bvb", [128, 512], F32, kind="ExternalInput")
    d["y"] = nc.dram_tensor("y", [CIN, N], F16, kind="ExternalOutput")

    with tile.TileContext(nc) as tc:
        with (
            tc.tile_pool(name="sb", bufs=1) as sb,
            tc.tile_pool(name="ps", bufs=4, space="PSUM") as ps,
        ):
            for i in range(loop):
                if i:
                    with tc.tile_critical():
                        nc.all_engine_barrier()
                _emit(tc, d, sb, ps)
    nc.compile()
    return nc


_NC_CACHE = {}


def get_nc(loop=1):
    if loop not in _NC_CACHE:
        _NC_CACHE[loop] = _build(loop)
    return _NC_CACHE[loop]


def host_prep(x, W_fc, b_fc, W_last, b_last):
    """Full inputs -> list of 8 per-core input maps."""
    x = np.asarray(x, dtype=np.float32)
    W_fc = np.asarray(W_fc, dtype=np.float32)
    b_fc = np.asarray(b_fc, dtype=np.float32)
    W_last = np.asarray(W_last, dtype=np.float32)
    b_last = np.asarray(b_last, dtype=np.float32)

    hh = np.arange(H).repeat(D) * 3 * D  # 192h per f'=64h+d
    dd = np.tile(np.arange(D), H)
    pq, pk, pv = hh + dd, hh + D + dd, hh + 2 * D + dd

    wq = np.ascontiguousarray((W_fc[pq] * 0.125).T).astype(np.float16)
    wk = np.ascontiguousarray(W_fc[pk].T).astype(np.float16)
    wv = np.ascontiguousarray(W_fc[pv].T).astype(np.float16)
    wl = np.ascontiguousarray(W_last.T).astype(np.float16)
    bq, bk, bv = b_fc[pq] * 0.125, b_fc[pk], b_fc[pv]
    bqk = np.ascontiguousarray(
        np.concatenate([bq.reshape(4, 128).T, bk.reshape(4, 128).T], axis=1)
    ).astype(np.float32)
    bvb = np.ascontiguousarray(np.tile(bv[None, :], (128, 1))).astype(np.float32)

    xf = x.reshape(BS, CIN, N)
    maps = []
    for b in range(BS):
        maps.append(
            {
                "x16": xf[b].astype(np.float16),
                "xr16": (xf[b] + b_last[:, None]).astype(np.float16),
                "wq": wq,
                "wk": wk,
                "wv": wv,
                "wl": wl,
                "bqk": bqk,
                "bvb": bvb,
            }
        )
    return maps


def kernel(x, W_fc, b_fc, W_last, b_last):
    nc = get_nc()
    maps = host_prep(x, W_fc, b_fc, W_last, b_last)
    res = run_bass_kernel_spmd(nc, maps, core_ids=list(range(BS)))
    y = np.stack([res.results[b]["y"] for b in range(BS)])
    return y.astype(np.float32).reshape(BS, CIN, 32, 32)
